# revision 1
# baseline (speedup 1.0000x reference)
"""Multi-head attention (pre-LN + residual) on 8 trn2 NeuronCores.

Sharding: core r = (batch b = r//4, head group i = r%4, 4 heads each).
Per core: LN(x_b) -> x_norm^T (bf16) -> Q^T/K^T/V for its heads ->
scores^T = K Q^T per head pair (row-group packed) -> exp on ScalarE ->
AV matmul with ones-column (denominator for free) -> normalize ->
two 8-way AllToAlls (one per head pair, first overlapped with the
second pair's attention); each core ends with full-d attended^T for a
256-row seq slice of BOTH batches -> w_o matmul + bias + residual.
"""

import sys

sys.path.insert(0, "/opt/trn_rl_repo")

import numpy as np
import ml_dtypes

BF16 = ml_dtypes.bfloat16

# Problem constants (hardcoded per contract)
B = 2
S = 2048
D = 1024
H = 16
DK = 64
NCORES = 8
HLOC = 4  # heads per core
DLOC = HLOC * DK  # 256
SLICE = S // NCORES  # 256 output rows per batch per core
EPS = 1e-5
SCALE = 1.0 / np.sqrt(DK)

_CACHE = {}


def _build(phases="ABCDE"):
    import concourse.bass as bass
    import concourse.mybir as mybir
    import concourse.tile as tile
    from concourse import bacc
    from concourse.masks import make_identity

    dt = mybir.dt
    AF = mybir.ActivationFunctionType
    OP = mybir.AluOpType

    nc = bacc.Bacc(
        "TRN2",
        target_bir_lowering=False,
        debug=False,
        enable_asserts=False,
        num_devices=NCORES,
    )

    # ---- I/O ----
    x_b = nc.dram_tensor("x_b", [S, D], dt.float32, kind="ExternalInput").ap()
    wqT = nc.dram_tensor("wqT", [D, DLOC], dt.bfloat16, kind="ExternalInput").ap()
    wkT = nc.dram_tensor("wkT", [D, DLOC], dt.bfloat16, kind="ExternalInput").ap()
    wvT = nc.dram_tensor("wvT", [D, DLOC], dt.bfloat16, kind="ExternalInput").ap()
    woT = nc.dram_tensor("woT", [D, D], dt.bfloat16, kind="ExternalInput").ap()
    x_res = nc.dram_tensor(
        "x_res", [B, SLICE, D], dt.float32, kind="ExternalInput"
    ).ap()
    b_o = nc.dram_tensor("b_o", [D], dt.float32, kind="ExternalInput").ap()
    gamma = nc.dram_tensor("gamma", [D], dt.float32, kind="ExternalInput").ap()
    beta = nc.dram_tensor("beta", [D], dt.float32, kind="ExternalInput").ap()
    out_sl = nc.dram_tensor(
        "out_sl", [B, SLICE, D], dt.float32, kind="ExternalOutput"
    ).ap()

    ST = S // 128  # 16 seq tiles
    FT = D // 128  # 8 feature tiles
    QC = S // 512  # 4 q-chunks for attention
    RT = B * SLICE // 128  # 4 row tiles of the output slice

    with tile.TileContext(nc) as tc:
        with (
            tc.tile_pool(name="singles", bufs=1) as singles,
            tc.tile_pool(name="persist", bufs=1) as persist,
            tc.tile_pool(name="dram", bufs=1, space="DRAM") as dram,
        ):
            # ---- constants needed by phase A ----
            # (ln_gamma is folded into the host-side QKV weights; ln_beta
            # is asserted zero host-side)
            ident = singles.tile([128, 128], dt.bfloat16)
            make_identity(nc, ident)
            eps_t = singles.tile([128, 1], dt.float32)
            nc.vector.memset(eps_t, EPS)

            # ---- persistent intermediates ----
            xnt_t = persist.tile([128, FT, S], dt.bfloat16, tag="xnt", name="xnt")
            xnt = [xnt_t[:, f, :] for f in range(FT)]
            qT = [
                persist.tile([128, S], dt.bfloat16, tag=f"qT{m}", name=f"qT{m}")
                for m in range(2)
            ]
            kT = [
                persist.tile([128, S], dt.bfloat16, tag=f"kT{m}", name=f"kT{m}")
                for m in range(2)
            ]
            vp_t = persist.tile(
                [128, HLOC, ST, DK + 1], dt.bfloat16, tag="vp", name="vp"
            )
            vp = [vp_t[:, h, :, :] for h in range(HLOC)]
            nc.gpsimd.memset(vp_t[:, :, :, DK : DK + 1], 1.0)

            # collective bounce buffers, one pair per head pair
            a2a_in = [
                dram.tile([NCORES, 2 * DK, SLICE], dt.bfloat16, name=f"a2a_in{m}", tag=f"a2a_in{m}")
                for m in range(2)
            ]
            a2a_out = [
                dram.tile([NCORES, 2 * DK, SLICE], dt.bfloat16, name=f"a2a_out{m}", tag=f"a2a_out{m}")
                for m in range(2)
            ]

            # QKV weights (loaded after the first LN tiles are in flight)
            wq_sb = singles.tile([128, FT, DLOC], dt.bfloat16)
            wk_sb = singles.tile([128, FT, DLOC], dt.bfloat16)
            wv_sb = singles.tile([128, FT, DLOC], dt.bfloat16)
            b_bc = singles.tile([128, D], dt.float32)
            wo_sb = singles.tile([128, FT, D], dt.bfloat16)
            xrb = singles.tile([128, RT, D], dt.float32)

            # ===== Phases A-C, software-pipelined =========================
            # Block c: LN seq tiles 4c..4c+3 -> K/Q (seq chunk c) + V ->
            # attention kt-steps 4c..4c+3 of q-chunk 0 (both head pairs).
            # Afterwards: remaining q-chunks head-pair-major, with each
            # pair's AllToAll fired as soon as the pair completes.
            x_rows = x_b.rearrange("(t p) d -> t p d", p=128)
            with (
                tc.tile_pool(name="ln", bufs=6) as ln_pool,
                tc.tile_pool(name="lnst", bufs=6) as lnst,
                tc.tile_pool(name="epool", bufs=6) as epool,
                tc.tile_pool(name="aopool", bufs=4) as aopool,
                tc.tile_pool(name="ivpool", bufs=4) as ivpool,
            ):

                def emit_ln(st):
                    x_t = ln_pool.tile([128, D], dt.float32, tag="x", name="x_t")
                    nc.sync.dma_start(out=x_t, in_=x_rows[st])
                    stats = lnst.tile([128, 2, 6], dt.float32, tag="stats", name="stats")
                    for g in range(2):
                        nc.vector.bn_stats(
                            out=stats[:, g, :], in_=x_t[:, g * 512 : (g + 1) * 512]
                        )
                    mv = lnst.tile([128, 2], dt.float32, tag="mv", name="mv")
                    nc.vector.bn_aggr(out=mv, in_=stats)
                    sd = lnst.tile([128, 1], dt.float32, tag="sd", name="sd")
                    nc.scalar.activation(
                        out=sd, in_=mv[:, 1:2], func=AF.Sqrt, bias=eps_t, scale=1.0
                    )
                    rinv = lnst.tile([128, 1], dt.float32, tag="rinv", name="rinv")
                    nc.vector.reciprocal_approx_fast(out=rinv, in_=sd)
                    # -mu * rinv, so ACT can do (x - mu) * rinv in one pass
                    negmur = lnst.tile([128, 1], dt.float32, tag="negmur", name="negmur")
                    nc.vector.tensor_scalar(
                        out=negmur,
                        in0=mv[:, 0:1],
                        scalar1=rinv,
                        scalar2=-1.0,
                        op0=OP.mult,
                        op1=OP.mult,
                    )
                    xn = ln_pool.tile([128, D], dt.bfloat16, tag="xn", name="xn")
                    nc.scalar.activation(
                        out=xn, in_=x_t, func=AF.Identity, bias=negmur, scale=rinv
                    )
                    for fp in range(FT // 2):
                        tr_ps = ps_tr.tile([128, 256], dt.bfloat16, tag="tr", name="tr")
                        for k in range(2):
                            nc.tensor.transpose(
                                tr_ps[:, k * 128 : (k + 1) * 128],
                                xn[:, (2 * fp + k) * 128 : (2 * fp + k + 1) * 128],
                                ident,
                            )
                        dst = xnt_t[:, 2 * fp : 2 * fp + 2, st * 128 : (st + 1) * 128]
                        src_v = tr_ps.rearrange("p (k c) -> p k c", k=2)
                        if fp % 2 == 0:
                            nc.vector.tensor_copy(out=dst, in_=src_v)
                        else:
                            nc.scalar.copy(out=dst, in_=src_v)

                def emit_kq(w_sb, dst, mt, ch):
                    ps = ps_tr.tile([128, 512], dt.float32, tag="qkv", name="kq_ps")
                    for fc in range(FT):
                        nc.tensor.matmul(
                            ps,
                            lhsT=w_sb[:, fc, mt * 128 : (mt + 1) * 128],
                            rhs=xnt[fc][:, ch * 512 : (ch + 1) * 512],
                            start=(fc == 0),
                            stop=(fc == FT - 1),
                        )
                    nc.vector.tensor_copy(
                        out=dst[mt][:, ch * 512 : (ch + 1) * 512], in_=ps
                    )

                def emit_v(st):
                    ps = ps_tr.tile([128, DLOC], dt.float32, tag="qkv", name="v_ps")
                    for fc in range(FT):
                        nc.tensor.matmul(
                            ps,
                            lhsT=xnt[fc][:, st * 128 : (st + 1) * 128],
                            rhs=wv_sb[:, fc, :],
                            start=(fc == 0),
                            stop=(fc == FT - 1),
                        )
                    nc.vector.tensor_copy(
                        out=vp_t[:, :, st, 0:DK],
                        in_=ps.rearrange("p (h d) -> p h d", h=HLOC),
                    )

                def emit_scores(hp, qc, kt):
                    s_ps = ps_s.tile([128, 1024], dt.float32, tag="s", name="s_ps")
                    for j in range(2):
                        nc.tensor.matmul(
                            s_ps[:, j * 512 : (j + 1) * 512],
                            lhsT=kT[hp][
                                j * 64 : (j + 1) * 64, kt * 128 : (kt + 1) * 128
                            ],
                            rhs=qT[hp][
                                j * 64 : (j + 1) * 64, qc * 512 : (qc + 1) * 512
                            ],
                            start=True,
                            stop=True,
                        )
                    e_t = epool.tile([128, 1024], dt.bfloat16, tag="e", name="e_t")
                    nc.scalar.activation(
                        out=e_t, in_=s_ps, func=AF.Exp, scale=float(SCALE)
                    )
                    return e_t

                def emit_av(hp, kt, av, e_t):
                    for j in range(2):
                        nc.tensor.matmul(
                            av[j],
                            lhsT=vp[2 * hp + j][:, kt, :],
                            rhs=e_t[:, j * 512 : (j + 1) * 512],
                            start=(kt == 0),
                            stop=(kt == ST - 1),
                        )

                def alloc_av(hp):
                    return [
                        ps_av.tile(
                            [DK + 1, 512],
                            dt.float32,
                            tag=f"av{j}",
                            name=f"av{hp}{j}",
                        )
                        for j in range(2)
                    ]

                def emit_normalize(hp, qc, av):
                    for j in range(2):
                        # single quick eviction frees the PSUM slot for the
                        # next q-chunk; the normalize chain runs off SBUF
                        avs = aopool.tile(
                            [DK, 512], dt.float32, tag="avs", name="avs"
                        )
                        nc.vector.tensor_copy(out=avs, in_=av[j][0:DK, :])
                        den = ivpool.tile([1, 512], dt.float32, tag="den", name="den")
                        nc.vector.tensor_copy(out=den, in_=av[j][DK : DK + 1, :])
                        invd = ivpool.tile(
                            [1, 512], dt.float32, tag="invd", name="invd"
                        )
                        nc.vector.reciprocal_approx_fast(out=invd, in_=den)
                        ibc = ivpool.tile([DK, 512], dt.float32, tag="ibc", name="ibc")
                        nc.gpsimd.partition_broadcast(ibc, invd)
                        ao = aopool.tile([DK, 512], dt.bfloat16, tag="ao", name="ao")
                        nc.vector.tensor_tensor(
                            out=ao, in0=avs, in1=ibc, op=OP.mult
                        )
                        for half in range(2):
                            nc.sync.dma_start(
                                out=a2a_in[hp][
                                    2 * qc + half, j * DK : (j + 1) * DK, :
                                ],
                                in_=ao[:, half * 256 : (half + 1) * 256],
                            )

                # -- block loop: LN -> K/Q (chunk c) -> V ---------------
                ps_tr_cm = tc.tile_pool(name="ps_tr", bufs=2, space="PSUM")
                ps_tr = ps_tr_cm.__enter__()
                for c in range(4):
                    for st in range(4 * c, 4 * c + 4):
                        emit_ln(st)
                    if c == 0:
                        nc.sync.dma_start(
                            out=wk_sb, in_=wkT.rearrange("(t p) m -> p t m", p=128)
                        )
                        nc.sync.dma_start(
                            out=wq_sb, in_=wqT.rearrange("(t p) m -> p t m", p=128)
                        )
                        nc.sync.dma_start(
                            out=wv_sb, in_=wvT.rearrange("(t p) m -> p t m", p=128)
                        )
                    for mt in range(2):
                        emit_kq(wk_sb, kT, mt, c)
                    for mt in range(2):
                        emit_kq(wq_sb, qT, mt, c)
                    for st in range(4 * c, 4 * c + 4):
                        emit_v(st)
                    if c == 1:
                        # heavy constants: behind the first blocks in priority
                        nc.sync.dma_start(
                            out=b_bc,
                            in_=bass.AP(
                                tensor=b_o.tensor,
                                offset=b_o.offset,
                                ap=[[0, 128]] + list(b_o.ap),
                            ),
                        )
                        nc.sync.dma_start(
                            out=wo_sb, in_=woT.rearrange("(t p) m -> p t m", p=128)
                        )
                        nc.sync.dma_start(
                            out=xrb,
                            in_=x_res.rearrange("b (t p) d -> p (b t) d", p=128),
                        )
                        for t in range(RT):
                            nc.vector.tensor_add(
                                out=xrb[:, t, :], in0=xrb[:, t, :], in1=b_bc
                            )
                ps_tr_cm.__exit__(None, None, None)

                # -- attention, head-pair-major + split AllToAll --------
                ps_s_cm = tc.tile_pool(name="ps_s", bufs=3, space="PSUM")
                ps_s = ps_s_cm.__enter__()
                ps_av_cm = tc.tile_pool(name="ps_av", bufs=1, space="PSUM")
                ps_av = ps_av_cm.__enter__()
                for hp in range(2):
                    for qc in range(QC):
                        av = alloc_av(hp)
                        for kt in range(ST):
                            e_t = emit_scores(hp, qc, kt)
                            emit_av(hp, kt, av, e_t)
                        emit_normalize(hp, qc, av)
                    if "D" in phases:
                        nc.gpsimd.collective_compute(
                            "AllToAll",
                            mybir.AluOpType.bypass,
                            replica_groups=[list(range(NCORES))],
                            ins=[a2a_in[hp].opt()],
                            outs=[a2a_out[hp].opt()],
                        )

                ps_av_cm.__exit__(None, None, None)
                ps_s_cm.__exit__(None, None, None)

            # ============ Phase E: output projection ======================
            # gathered slot r of a2a_out[hp] = heads {4i+2hp, 4i+2hp+1} of
            # group i = r%4, batch r//4 -> f-tile index 2*(r%4) + hp
            if "E" in phases:
                with (
                    tc.tile_pool(name="ps_wo", bufs=1, space="PSUM") as ps_wo,
                    tc.tile_pool(name="attg", bufs=1) as attg_pool,
                    tc.tile_pool(name="outp", bufs=4) as outp,
                ):
                    wo_ps = {}
                    for b in range(B):
                        for mt in range(SLICE // 128):
                            for oc in range(2):
                                wo_ps[b, mt, oc] = ps_wo.tile(
                                    [128, 512],
                                    dt.float32,
                                    tag=f"wo{b}{mt}{oc}",
                                    name=f"wo{b}{mt}{oc}",
                                )
                    attg = {}
                    for hp in range(2):
                        for b in range(B):
                            ag = attg_pool.tile(
                                [128, 4, SLICE],
                                dt.bfloat16,
                                tag=f"ag{hp}{b}",
                                name=f"ag{hp}{b}",
                            )
                            attg[hp, b] = ag
                            nc.sync.dma_start(
                                out=ag,
                                in_=a2a_out[hp][4 * b : 4 * (b + 1), :, :].rearrange(
                                    "s (t p) q -> p (s t) q", p=128
                                ),
                            )
                        for b in range(B):
                            for mt in range(SLICE // 128):
                                for oc in range(2):
                                    for i4 in range(4):
                                        nc.tensor.matmul(
                                            wo_ps[b, mt, oc],
                                            lhsT=attg[hp, b][
                                                :, i4, mt * 128 : (mt + 1) * 128
                                            ],
                                            rhs=wo_sb[
                                                :,
                                                2 * i4 + hp,
                                                oc * 512 : (oc + 1) * 512,
                                            ],
                                            start=(hp == 0 and i4 == 0),
                                            stop=(hp == 1 and i4 == 3),
                                        )
                    for b in range(B):
                        for mt in range(SLICE // 128):
                            for oc in range(2):
                                o_t = outp.tile([128, 512], dt.float32, tag="o")
                                nc.vector.tensor_tensor(
                                    out=o_t,
                                    in0=wo_ps[b, mt, oc],
                                    in1=xrb[
                                        :,
                                        b * (SLICE // 128) + mt,
                                        oc * 512 : (oc + 1) * 512,
                                    ],
                                    op=OP.add,
                                )
                                nc.sync.dma_start(
                                    out=out_sl[
                                        b,
                                        mt * 128 : (mt + 1) * 128,
                                        oc * 512 : (oc + 1) * 512,
                                    ],
                                    in_=o_t,
                                )
            else:
                nc.sync.dma_start(out=out_sl[:, :, :], in_=x_res[:, :, :])

    nc.compile()
    return nc


def _get_nc(phases="ABCDE"):
    key = ("nc", phases)
    if key not in _CACHE:
        _CACHE[key] = _build(phases)
    return _CACHE[key]


def _make_in_maps(inputs):
    x = np.asarray(inputs["x"], np.float32)
    w_q = np.asarray(inputs["w_q"], np.float32)
    w_k = np.asarray(inputs["w_k"], np.float32)
    w_v = np.asarray(inputs["w_v"], np.float32)
    w_o = np.asarray(inputs["w_o"], np.float32)
    b_o = np.asarray(inputs["b_o"], np.float32)
    gamma = np.asarray(inputs["ln_gamma"], np.float32)
    beta = np.asarray(inputs["ln_beta"], np.float32)

    assert np.allclose(beta, 0.0), "nonzero ln_beta not supported"
    woT = np.ascontiguousarray(w_o.T).astype(BF16)
    # LN gamma folds exactly into the input side of the QKV projections
    w_qg = w_q * gamma[None, :]
    w_kg = w_k * gamma[None, :]
    w_vg = w_v * gamma[None, :]
    in_maps = []
    for r in range(NCORES):
        b, i = r // 4, r % 4
        sl = slice(DLOC * i, DLOC * (i + 1))
        in_maps.append(
            {
                "x_b": np.ascontiguousarray(x[b]),
                "wqT": np.ascontiguousarray(w_qg[sl].T).astype(BF16),
                "wkT": np.ascontiguousarray(w_kg[sl].T).astype(BF16),
                "wvT": np.ascontiguousarray(w_vg[sl].T).astype(BF16),
                "woT": woT,
                "x_res": np.ascontiguousarray(x[:, SLICE * r : SLICE * (r + 1), :]),
                "b_o": b_o,
                "gamma": gamma,
                "beta": beta,
            }
        )
    return in_maps


def _install_ntff_hook():
    """The agent image's antenv lacks axon_hooks; recreate it so
    trace=True can capture NTFF profiles through libaxon_pjrt.so."""
    import types

    from concourse import bass_utils

    if "antenv.axon_hooks" not in sys.modules:
        import antenv
        from trn_agent_boot.trn_boot import _ntff_profile_via_ctypes

        mod = types.ModuleType("antenv.axon_hooks")
        state = {}
        mod.set_axon_ntff_profile_hook = lambda h: state.update(h=h)
        mod.get_axon_ntff_profile_hook = lambda: state.get("h")
        sys.modules["antenv.axon_hooks"] = mod
        antenv.axon_hooks = mod
        mod.set_axon_ntff_profile_hook(
            _ntff_profile_via_ctypes("/opt/axon/libaxon_pjrt.so")
        )
    # no bucket access in this container; keep artifacts local
    bass_utils.upload_artifacts = lambda tmpdir: tmpdir


def run(inputs, trace=False, phases="ABCDE", tmpdir=None, trace_cores=None):
    from concourse import bass_utils

    if trace:
        _install_ntff_hook()
    nc = _get_nc(phases)
    in_maps = _make_in_maps(inputs)
    res = bass_utils.run_bass_kernel_spmd(
        nc,
        in_maps,
        core_ids=list(range(NCORES)),
        trace=trace,
        tmpdir=tmpdir,
        trace_cores=trace_cores,
    )
    out = np.empty((B, S, D), np.float32)
    for r in range(NCORES):
        out[:, SLICE * r : SLICE * (r + 1), :] = res.results[r]["out_sl"]
    return out, res


def kernel(**inputs):
    out, _ = run(inputs)
    return out

